# revision 2
# baseline (speedup 1.0000x reference)
"""Trainium2 Bass kernel for RoPE linear attention (no softmax, strict causal).

Chunked linear attention, C=256. See kernel_v2 docstring for the math.
v3: batched DMAs (tables in 2, Q per group, out per group), group-wide rope
ops, full-bank transpose PSUM, loads+rope emitted one group ahead of compute.
"""

import math
import os
import sys

import numpy as np

for _p in ("/opt/trn_rl_repo",):
    if _p not in sys.path and os.path.isdir(_p):
        sys.path.insert(0, _p)

THETA = 2 ** 16
B, H, T, N, D = 2, 8, 2048, 1024, 128
NB = T // 128            # 16 t-blocks
NG = NB // 2             # 8 groups of 2 blocks (C=256)
NCHUNK = N // 128        # 8 n-chunks
NC_COUNT = 8
BH_PER_CORE = (B * H) // NC_COUNT  # 2

_cache = {}


def _make_tables():
    """Interleaved cos/sin tables [T, N] fp16, sign-folded for the swap AP."""
    import jax
    import jax.numpy as jnp

    with jax.default_device(jax.devices("cpu")[0]):
        pos = jnp.floor(jnp.arange(N, dtype=jnp.float32) / 2.0) * 2.0
        freqs = 1.0 / (THETA ** (pos / N)) / (2.0 * math.pi)
        r_phases = jnp.arange(T, dtype=jnp.float32)[:, None] * freqs[None, :]
        ph = (r_phases % 1.0) * (2.0 * math.pi)
        c = np.asarray(jnp.cos(ph))          # (T, N) fp32
        s = np.asarray(jnp.sin(ph))
    alt = np.where(np.arange(N) % 2 == 0, -1.0, 1.0).astype(np.float32)
    return c.astype(np.float16), (s * alt[None, :]).astype(np.float16)


def _build_nc():
    import concourse.mybir as mybir
    from concourse import bacc
    from concourse.tile import TileContext

    f32 = mybir.dt.float32
    f16 = mybir.dt.float16

    ct_np, st_np = _make_tables()
    mask_np = np.triu(np.ones((128, 128), np.float16), 1)  # keep s < t
    ident_np = np.eye(128, dtype=np.float16)

    nc = bacc.Bacc("TRN2", target_bir_lowering=False, debug=False,
                   num_devices=NC_COUNT)
    q = nc.dram_tensor("q", [BH_PER_CORE, T, N], f32, kind="ExternalInput")
    v = nc.dram_tensor("v", [BH_PER_CORE, T, D], f32, kind="ExternalInput")
    out = nc.dram_tensor("out", [BH_PER_CORE, T, D], f32,
                         kind="ExternalOutput")
    ct_dram = nc.inline_tensor(ct_np, name="ct_tab")
    st_dram = nc.inline_tensor(st_np, name="st_tab")
    mask_dram = nc.inline_tensor(mask_np, name="mask_tab")
    ident_dram = nc.inline_tensor(ident_np, name="ident_tab")

    with TileContext(nc) as tc:
        with tc.tile_pool(name="const", bufs=1) as cpool, \
             tc.tile_pool(name="work", bufs=1) as pool, \
             tc.tile_pool(name="psT", bufs=2, space="PSUM") as psT, \
             tc.tile_pool(name="psS", bufs=1, space="PSUM") as psS, \
             tc.tile_pool(name="psO", bufs=1, space="PSUM") as psO, \
             tc.tile_pool(name="psM", bufs=1, space="PSUM") as psM:

            mask_sb = cpool.tile([128, 128], f16, name="mask")
            nc.sync.dma_start(out=mask_sb, in_=mask_dram[:, :])
            ident_sb = cpool.tile([128, 128], f16, name="ident")
            nc.sync.dma_start(out=ident_sb, in_=ident_dram[:, :])

            # V tiles allocated now, casting DMAs emitted in quarters later
            # (so they don't delay the first Q groups on the gpsimd queue)
            vf = [cpool.tile([128, NB * 128], f16, name=f"vf{bh}")
                  for bh in range(BH_PER_CORE)]

            def load_v_quarter(bh, qtr):
                nb4 = NB // 4
                sl = slice(qtr * nb4 * 128, (qtr + 1) * nb4 * 128)
                nc.gpsimd.dma_start(
                    out=vf[bh][:, sl].rearrange("p (a d) -> p a d", a=nb4),
                    in_=v[bh, sl].rearrange("(a p) d -> p a d", p=128),
                )

            # tables: per-group chunk DMAs so rope(g) starts early
            ct_all = cpool.tile([128, NB * N], f16, name="ct_all")
            st_all = cpool.tile([128, NB * N], f16, name="st_all")

            def load_tab_chunk(j):       # group j = blocks 2j, 2j+1
                sl_sb = slice(2 * j * N, (2 * j + 2) * N)
                sl_dr = slice(2 * j * 128, (2 * j + 2) * 128)
                for sb, dr in ((ct_all, ct_dram), (st_all, st_dram)):
                    nc.sync.dma_start(
                        out=sb[:, sl_sb].rearrange("p (a n) -> p a n", a=2),
                        in_=dr[sl_dr].rearrange("(a p) n -> p a n", p=128))

            # M state: long-lived PSUM accumulators (2 banks per bh)
            mps = [psM.tile([128, N], f32, tag=f"m{bh}", name=f"mps{bh}")
                   for bh in range(BH_PER_CORE)]
            m_sb = [[cpool.tile([128, N], f16, name=f"msb{bh}_{i}")
                     for i in range(2)] for bh in range(BH_PER_CORE)]

            qd = [[None] * NG for _ in range(BH_PER_CORE)]

            def load_rope(bh, g):
                """Group Q load (casting DMA) + group-wide 3-op rope."""
                t = pool.tile([128, 2 * N], f16, tag="qd", bufs=4,
                              name=f"qd{bh}_{g}")
                nc.gpsimd.dma_start(
                    out=t.rearrange("p (a n) -> p a n", a=2),
                    in_=q[bh, g * 256:(g + 1) * 256, :]
                        .rearrange("(a p) n -> p a n", p=128))
                qd[bh][g] = t
                c_sl = ct_all[:, 2 * g * N:(2 * g + 2) * N]
                s_sl = st_all[:, 2 * g * N:(2 * g + 2) * N]
                t3 = t.rearrange("p (a b) -> p a b", b=2)
                tsw = t3[:, :, ::-1]
                c3 = c_sl.rearrange("p (a b) -> p a b", b=2)
                s3 = s_sl.rearrange("p (a b) -> p a b", b=2)
                u = pool.tile([128, 2 * N], f16, tag="ropeu", bufs=2,
                              name=f"u{bh}_{g}")
                w = pool.tile([128, 2 * N], f16, tag="ropew", bufs=2,
                              name=f"w{bh}_{g}")
                u3 = u.rearrange("p (a b) -> p a b", b=2)
                w3 = w.rearrange("p (a b) -> p a b", b=2)
                nc.vector.tensor_mul(out=u3, in0=tsw, in1=s3)
                nc.vector.tensor_mul(out=w3, in0=t3, in1=c3)
                nc.vector.tensor_add(out=t, in0=w, in1=u)

            qrt = [[None] * NG for _ in range(BH_PER_CORE)]
            psx_t = [[None] * NG for _ in range(BH_PER_CORE)]
            pstrip = [[None] * NG for _ in range(BH_PER_CORE)]
            pox_t = [[None] * NG for _ in range(BH_PER_CORE)]

            def phase_transpose(bh, g):
                """PE transposes of both blocks + one qrt drain per block."""
                qd_g = qd[bh][g]
                qrt_g = pool.tile([128, NCHUNK * 256], f16, tag="qrt",
                                  bufs=3, name=f"qrt{bh}_{g}")
                qrt[bh][g] = qrt_g
                qrt3 = qrt_g.rearrange("p (c t) -> p c t", c=NCHUNK)
                for bi in range(2):
                    src = qd_g[:, bi * N:(bi + 1) * N]
                    pt = psT.tile([128, N], f16, tag="pt",
                                  name=f"pt{bh}_{g}_{bi}")
                    for k in range(NCHUNK):
                        nc.tensor.transpose(
                            pt[:, k * 128:(k + 1) * 128],
                            src[:, k * 128:(k + 1) * 128],
                            ident_sb)
                    nc.scalar.copy(
                        qrt3[:, :, bi * 128:(bi + 1) * 128],
                        pt.rearrange("p (c t) -> p c t", c=NCHUNK))

            def phase_intra(bh, g):
                # intra scores (one bank: ps0 cols 0:256, ps1 256:384).
                # NOTE: start=True clears has_written for the WHOLE bank, so
                # only the first matmul touching the psx bank carries it;
                # ps1's first write lands on cleared bits -> overwrite+set.
                qrt_g = qrt[bh][g]
                psx = psS.tile([128, 384], f32, tag="ps",
                               name=f"psx_{bh}_{g}")
                psx_t[bh][g] = psx
                ps0 = psx[:, 0:256]
                ps1 = psx[:, 256:384]
                for k in range(NCHUNK):
                    ka = qrt_g[:, k * 256:k * 256 + 128]
                    kfull = qrt_g[:, k * 256:(k + 1) * 256]
                    kb = qrt_g[:, k * 256 + 128:(k + 1) * 256]
                    nc.tensor.matmul(ps0, lhsT=ka, rhs=kfull,
                                     start=(k == 0), stop=(k == NCHUNK - 1))
                    nc.tensor.matmul(ps1, lhsT=kb, rhs=kb,
                                     start=False, stop=(k == NCHUNK - 1))

            def phase_pdrain(bh, g):
                """P strip drains (AV lhsT, [s,t] layout fp16)."""
                psx = psx_t[bh][g]
                ps0 = psx[:, 0:256]
                ps1 = psx[:, 256:384]
                p00 = pool.tile([128, 128], f16, tag="p00", bufs=3,
                                name=f"p00_{bh}_{g}")
                p01 = pool.tile([128, 128], f16, tag="p01", bufs=3,
                                name=f"p01_{bh}_{g}")
                p11 = pool.tile([128, 128], f16, tag="p11", bufs=3,
                                name=f"p11_{bh}_{g}")
                nc.vector.tensor_mul(out=p00, in0=ps0[:, 0:128],
                                     in1=mask_sb)
                nc.scalar.copy(p01, ps0[:, 128:256])
                nc.vector.tensor_mul(out=p11, in0=ps1, in1=mask_sb)
                pstrip[bh][g] = (p00, p01, p11)

            def phase_mm(bh, g):
                """inter + update + AV matmuls (one pox bank-clear start)."""
                b0, b1 = 2 * g, 2 * g + 1
                qd_g = qd[bh][g]
                qrt_g = qrt[bh][g]
                p00, p01, p11 = pstrip[bh][g]
                msb_prev = m_sb[bh][(g + 1) % 2]   # state after group g-1
                pox = psO.tile([128, 2 * D], f32, tag="po",
                               name=f"pox_{bh}_{g}")
                pox_t[bh][g] = pox
                po0 = pox[:, 0:D]
                po1 = pox[:, D:2 * D]
                if g > 0:
                    for k in range(NCHUNK):
                        nc.tensor.matmul(
                            po0,
                            lhsT=qrt_g[:, k * 256:k * 256 + 128],
                            rhs=msb_prev[:, k * 128:(k + 1) * 128],
                            start=(k == 0), stop=False)
                        nc.tensor.matmul(
                            po1,
                            lhsT=qrt_g[:, k * 256 + 128:(k + 1) * 256],
                            rhs=msb_prev[:, k * 128:(k + 1) * 128],
                            start=False, stop=False)
                # update (not for last group: M_7 is never read).
                # mps spans 2 banks (chunks 0-3 / 4-7): exactly one
                # bank-clearing start per bank (k==0 and k==4 of the very
                # first block); all other writes rely on cleared bits.
                if g < NG - 1:
                    for bi, b in enumerate((b0, b1)):
                        src = qd_g[:, bi * N:(bi + 1) * N]
                        for k in range(NCHUNK):
                            nc.tensor.matmul(
                                mps[bh][:, k * 128:(k + 1) * 128],
                                lhsT=src[:, k * 128:(k + 1) * 128],
                                rhs=vf[bh][:, b * 128:(b + 1) * 128],
                                start=(b == 0 and g == 0 and k % 4 == 0),
                                stop=(g == NG - 2 and bi == 1),
                            )
                nc.tensor.matmul(po0, lhsT=p00,
                                 rhs=vf[bh][:, b0 * 128:(b0 + 1) * 128],
                                 start=(g == 0), stop=True)
                nc.tensor.matmul(po1, lhsT=p01,
                                 rhs=vf[bh][:, b0 * 128:(b0 + 1) * 128],
                                 start=False, stop=False)
                nc.tensor.matmul(po1, lhsT=p11,
                                 rhs=vf[bh][:, b1 * 128:(b1 + 1) * 128],
                                 start=False, stop=True)

            def phase_drain(bh, g):
                """M drain + out drain (ACT) + out DMA (sync)."""
                b0, b1 = 2 * g, 2 * g + 1
                if g < NG - 1:
                    nc.scalar.copy(m_sb[bh][g % 2], mps[bh])
                ob = pool.tile([128, 2 * D], f32, tag="ostage", bufs=3,
                               name=f"ob{bh}_{g}")
                nc.scalar.copy(ob, pox_t[bh][g])
                nc.sync.dma_start(
                    out=out[bh, b0 * 128:(b1 + 1) * 128, :]
                        .rearrange("(a p) d -> p a d", p=128),
                    in_=ob.rearrange("p (a d) -> p a d", a=2))

            # prologue: tables for groups 0/1, Q+rope for groups 0/1,
            # first V quarter
            load_tab_chunk(0)
            load_tab_chunk(1)
            for bh in range(BH_PER_CORE):
                load_rope(bh, 0)
            for bh in range(BH_PER_CORE):
                load_v_quarter(bh, 0)
            for bh in range(BH_PER_CORE):
                load_rope(bh, 1)

            # steady state: compute group g while loading+roping group g+2.
            # Phase interleave keeps every in-order queue free of
            # head-of-line blocking: all qrt drains first on ACT, masks
            # early on DVE (between the two rope emissions), stall-prone
            # M/out drains last.
            for g in range(NG):
                if g + 2 < NG:
                    load_tab_chunk(g + 2)
                for bh in range(BH_PER_CORE):
                    phase_transpose(bh, g)
                for bh in range(BH_PER_CORE):
                    phase_intra(bh, g)
                if g + 2 < NG:
                    load_rope(0, g + 2)
                for bh in range(BH_PER_CORE):
                    phase_pdrain(bh, g)
                if g + 2 < NG:
                    load_rope(1, g + 2)
                if g < 3:
                    for bh in range(BH_PER_CORE):
                        load_v_quarter(bh, g + 1)
                for bh in range(BH_PER_CORE):
                    phase_mm(bh, g)
                    phase_drain(bh, g)

    nc.compile()
    return nc


def _get_nc():
    if "nc" not in _cache:
        _cache["nc"] = _build_nc()
    return _cache["nc"]


def kernel(Q, K, V):
    from concourse import bass_utils

    del K  # K is Q by construction
    Qr = np.ascontiguousarray(Q.reshape(B * H, T, N), dtype=np.float32)
    Vr = np.ascontiguousarray(V.reshape(B * H, T, D), dtype=np.float32)

    nc = _get_nc()
    in_maps = []
    for c in range(NC_COUNT):
        lo = c * BH_PER_CORE
        in_maps.append({
            "q": np.ascontiguousarray(Qr[lo:lo + BH_PER_CORE]),
            "v": np.ascontiguousarray(Vr[lo:lo + BH_PER_CORE]),
        })

    res = bass_utils.run_bass_kernel_spmd(
        nc, in_maps, core_ids=list(range(NC_COUNT)),
    )
    _cache["last_result"] = res
    outs = [res.results[c]["out"].reshape(BH_PER_CORE, T, D)
            for c in range(NC_COUNT)]
    return np.concatenate(outs, axis=0).reshape(B, H, T, D).astype(np.float32)
